# revision 12
# baseline (speedup 1.0000x reference)
"""Causal multi-head attention block (B=4, S=1024, E=1024, H=16, D=64) on 8 TRN2 cores.

Sharding: data-parallel over batch (4) x tensor-parallel over heads (2 groups of 8).
Core i handles batch i//2, head-group i%2. Each core computes its partial output
projection (row-parallel W_proj); the host sums the two TP partials per batch and
applies the (exact) bias corrections.

Device-side math per core (bf16 compute, f32 accumulate):
  qT = (Wq_g)^T x^T + bq_g          [512, 1024]  (head-major rows h*64+d)
  kT = (Wk_g)^T x^T + bk_g          [512, 1024]
  v  = x Wv_g                       [1024, 512]
  For each head h: PT[sk, sq] = exp((kT_h^T qT_h)/8) * causal_mask (lower blocks only)
  PV with lhsT = [ones(64) | v_h(64)]: pso rows 0-63 = denominator (replicated 64x
  by the matmul itself), rows 64-127 = unnormalized o2T_h.  o2T_h = pso[64:] *
  recip(pso[:64])  (all-DVE normalize; no partition broadcast needed)
  out_partial = o2T^T Wp_g          [1024, 1024] bf16
Host: out[b] = out_partial[2b] + out_partial[2b+1] + (bv_0 Wp_0 + bv_1 Wp_1 + b_proj)
(the v-bias term is exact because softmax rows sum to 1).

Scheduling notes (from perfetto analysis of the previous version):
 - input DMAs are split fine-grained, deps-first, and spread over 4 trigger
   engines (sync/gpsimd/vector/scalar) so the first QKV chain starts ~9us
   instead of ~18us.
 - v-chain PSUM evacuation is on VectorE (was ScalarE, where it queued behind
   exps, starved the PE >3.4us and tripped the HAM clock gate to half rate).
 - the last head pair's QK/exp is split by sq-half so the final exps overlap
   PV/proj matmuls instead of idling the PE into a second HAM re-throttle.
 - output partials are bf16 (halves the output-DMA tail), DMAed per-row-block
   on alternating queues.
"""

import numpy as np
import ml_dtypes

import concourse.bass as bass
import concourse.tile as tile
from concourse import bacc, mybir
from concourse.bass_utils import run_bass_kernel_spmd
from concourse.masks import make_upper_triangular

BF16 = mybir.dt.bfloat16
F32 = mybir.dt.float32

B, S, E = 4, 1024, 1024
H_TOT, D = 16, 64
NCORES = 8
HL = 8            # heads per core
JL = HL * D       # 512 local qkv dim
P = 128
ET = E // P       # 8 k-tiles over embed dim
JT = JL // P      # 4 partition-tiles over local qkv dim

_NC_CACHE = None


def build_nc():
    nc = bacc.Bacc()

    xT = nc.declare_dram_parameter("xT", [E, S], BF16, isOutput=False)
    wq = nc.declare_dram_parameter("wq", [E, JL], BF16, isOutput=False)
    wk = nc.declare_dram_parameter("wk", [E, JL], BF16, isOutput=False)
    wv = nc.declare_dram_parameter("wv", [E, JL], BF16, isOutput=False)
    wp = nc.declare_dram_parameter("wp", [JL, E], BF16, isOutput=False)
    bq = nc.declare_dram_parameter("bq", [P, JT], F32, isOutput=False)
    bk = nc.declare_dram_parameter("bk", [P, JT], F32, isOutput=False)
    out = nc.declare_dram_parameter("out", [S, E], BF16, isOutput=True)

    with tile.TileContext(nc) as tc:
        with (
            tc.tile_pool(name="singles", bufs=1) as singles,
            tc.tile_pool(name="pt", bufs=4) as pt_pool,
            tc.tile_pool(name="rec", bufs=2) as rec_pool,
            tc.tile_pool(name="outst", bufs=3) as out_pool,
            tc.tile_pool(name="ps_mm", bufs=2, space="PSUM") as ps_mm,
            tc.tile_pool(name="ps_l", bufs=2, space="PSUM") as ps_l,
            tc.tile_pool(name="ps_o", bufs=2, space="PSUM") as ps_o,
        ):
            # ---- static inputs -> SBUF.  Fine-grained, deps-first DMAs
            # spread over four trigger engines: the first QKV chain (wq j-cols
            # 0:256 + xT kt0) must land within ~2us of the preamble ending.
            xT_sb = singles.tile([P, ET, S], BF16)
            wq_sb = singles.tile([P, ET, JL], BF16)
            wk_sb = singles.tile([P, ET, JL], BF16)
            wv_sb = singles.tile([P, ET, JL], BF16)
            wp_sb = singles.tile([P, JT, E], BF16)
            bq_sb = singles.tile([P, JT], F32)
            bk_sb = singles.tile([P, JT], F32)
            xT_r = xT[:, :].rearrange("(o p) s -> p o s", p=P)
            wq_r = wq[:, :].rearrange("(o p) j -> p o j", p=P)
            wk_r = wk[:, :].rearrange("(o p) j -> p o j", p=P)
            wv_r = wv[:, :].rearrange("(o p) j -> p o j", p=P)

            # gpsimd: wq/wk chunks ordered by first use; the kt0 slivers
            # (64KB) land first so phase A's opening matmuls gate on the
            # minimum possible bytes.  wp (proj-only) goes last.
            nc.gpsimd.dma_start(out=wq_sb[:, 0:1, 0:256], in_=wq_r[:, 0:1, 0:256])
            nc.gpsimd.dma_start(out=wk_sb[:, 0:1, 0:256], in_=wk_r[:, 0:1, 0:256])
            nc.gpsimd.dma_start(out=wq_sb[:, 1:4, 0:256], in_=wq_r[:, 1:4, 0:256])
            nc.gpsimd.dma_start(out=wk_sb[:, 1:4, 0:256], in_=wk_r[:, 1:4, 0:256])
            nc.gpsimd.dma_start(out=wq_sb[:, 4:8, 0:256], in_=wq_r[:, 4:8, 0:256])
            nc.gpsimd.dma_start(out=wk_sb[:, 4:8, 0:256], in_=wk_r[:, 4:8, 0:256])
            nc.gpsimd.dma_start(out=wq_sb[:, :, 256:512], in_=wq_r[:, :, 256:512])
            nc.gpsimd.dma_start(out=wk_sb[:, :, 256:512], in_=wk_r[:, :, 256:512])
            nc.gpsimd.dma_start(out=wp_sb[:], in_=wp[:, :].rearrange("(o p) e -> p o e", p=P))
            # sync: x ktiles only, finest chunks first (phase A consumes them
            # kt-major at ~1.7us/ktile, matching the stream rate)
            nc.sync.dma_start(out=xT_sb[:, 0:1, 0:512], in_=xT_r[:, 0:1, 0:512])
            nc.sync.dma_start(out=xT_sb[:, 0:1, 512:1024], in_=xT_r[:, 0:1, 512:1024])
            nc.sync.dma_start(out=xT_sb[:, 1:2], in_=xT_r[:, 1:2])
            for c in range(2, ET, 2):
                nc.sync.dma_start(out=xT_sb[:, c:c + 2], in_=xT_r[:, c:c + 2])
            # scalar: bias rows, then wv (first needed by the v chains, much later)
            nc.scalar.dma_start(out=bq_sb[:], in_=bq[:, :])
            nc.scalar.dma_start(out=bk_sb[:], in_=bk[:, :])
            nc.scalar.dma_start(out=wv_sb[:, 0:4], in_=wv_r[:, 0:4])
            nc.scalar.dma_start(out=wv_sb[:, 4:8], in_=wv_r[:, 4:8])

            # causal keep-mask for diagonal PT blocks: 1 where sq >= sk else 0
            mask_sb = singles.tile([P, P], BF16)
            make_upper_triangular(nc, mask_sb[:], val=1.0, diag=True)

            # ---- PE warmup: ~10 junk matmuls on a zeroed tile keep the PE
            # busy through the input-DMA lead-in, so the HAM clock gate is
            # already at 8/8 (2.4 GHz) when the first real matmul issues
            # (cold matmuls run at 1.2 GHz and DMA-wait gaps keep resetting
            # the ~3.4us sustained-busy warmup window) ----
            junk_sb = singles.tile([P, 512], BF16)
            nc.vector.memset(junk_sb[:], 0.0)
            for i in range(10):
                psw = ps_mm.tile([P, 512], F32, tag="mm", name=f"warm_{i}")
                nc.tensor.matmul(psw[:], lhsT=junk_sb[:, 0:128], rhs=junk_sb[:],
                                 start=True, stop=True)

            # ---- QKV projections ----
            qT_sb = singles.tile([P, JT, S], BF16)   # row j = h*64+d, head-major
            kT_sb = singles.tile([P, JT, S], BF16)
            o2T_sb = singles.tile([P, JT, S], BF16)  # normalized attn out, same row layout
            # PV stationary operand per (sk-tile, head): [ones(64) | v(64)].
            # The ones half makes the matmul emit the softmax denominator
            # replicated across 64 PSUM partitions -> all-DVE normalize.
            vaug_sb = singles.tile([P, ET, HL, 2 * D], BF16)
            nc.vector.memset(vaug_sb[:, :, :, 0:D], 1.0)

            def emit_phase_a():
                # q/k projections for jt0+jt1 processed KT-MAJOR across 8
                # parallel PSUM accumulation groups: each x-ktile is consumed
                # by 8 matmuls (~1.7us) right as the next ktile streams in, so
                # the PE never starves during the input DMA lead-in (a starved
                # PE resets the HAM busy-window and stretches the cold-clock
                # warmup).  Bias-adds run on ScalarE, idle until the first exp.
                tiles = [ps_mm.tile([P, 512], F32, tag="mm", name="pa_mm0"),
                         ps_mm.tile([P, 512], F32, tag="mm", name="pa_mm1"),
                         ps_l.tile([P, 1024], F32, tag="psl", name="pa_l0"),
                         ps_l.tile([P, 1024], F32, tag="psl", name="pa_l1"),
                         ps_o.tile([P, 512], F32, tag="pso", name="pa_o0"),
                         ps_o.tile([P, 512], F32, tag="pso", name="pa_o1")]
                # jt0 groups live in the ps_l tiles and evacuate FIRST, so
                # QK pair0's psl allocations (same pool) unblock earliest
                aps = [tiles[2][:, 0:512], tiles[2][:, 512:1024],
                       tiles[3][:, 0:512], tiles[3][:, 512:1024],
                       tiles[0][:], tiles[1][:],
                       tiles[4][:], tiles[5][:]]
                groups = [
                    (wq_sb, bq_sb, qT_sb, 0, 0), (wk_sb, bk_sb, kT_sb, 0, 0),
                    (wq_sb, bq_sb, qT_sb, 0, 1), (wk_sb, bk_sb, kT_sb, 0, 1),
                    (wq_sb, bq_sb, qT_sb, 1, 0), (wk_sb, bk_sb, kT_sb, 1, 0),
                    (wq_sb, bq_sb, qT_sb, 1, 1), (wk_sb, bk_sb, kT_sb, 1, 1),
                ]
                for kt in range(ET):
                    for ap, (w_sb, _, _, jt, nb) in zip(aps, groups):
                        nc.tensor.matmul(
                            ap,
                            lhsT=w_sb[:, kt, jt * P:(jt + 1) * P],
                            rhs=xT_sb[:, kt, nb * 512:(nb + 1) * 512],
                            start=(kt == 0), stop=(kt == ET - 1),
                        )
                for ap, (_, b_sb, dst, jt, nb) in zip(aps, groups):
                    nc.scalar.activation(
                        out=dst[:, jt, nb * 512:(nb + 1) * 512], in_=ap,
                        func=mybir.ActivationFunctionType.Identity,
                        bias=b_sb[:, jt:jt + 1],
                    )

            def emit_qk_chains(jt):
                for w_sb, b_sb, dst in ((wq_sb, bq_sb, qT_sb), (wk_sb, bk_sb, kT_sb)):
                    # both 512-halves accumulate side by side so consecutive
                    # matmuls share the same stationary operand
                    pss = [ps_mm.tile([P, 512], F32, tag="mm", name=f"mm_{jt}_{nb}")
                           for nb in range(2)]
                    for kt in range(ET):
                        for nb in range(2):
                            nc.tensor.matmul(
                                pss[nb][:],
                                lhsT=w_sb[:, kt, jt * P:(jt + 1) * P],
                                rhs=xT_sb[:, kt, nb * 512:(nb + 1) * 512],
                                start=(kt == 0), stop=(kt == ET - 1),
                            )
                    for nb in range(2):
                        nc.vector.tensor_scalar_add(
                            dst[:, jt, nb * 512:(nb + 1) * 512], pss[nb][:],
                            b_sb[:, jt:jt + 1],
                        )

            def emit_v_chain(st):
                ps = ps_mm.tile([P, 512], F32, tag="mm", name=f"mmv_{st}")
                for kt in range(ET):
                    nc.tensor.matmul(
                        ps[:],
                        lhsT=xT_sb[:, kt, st * P:(st + 1) * P],
                        rhs=wv_sb[:, kt, :],
                        start=(kt == 0), stop=(kt == ET - 1),
                    )
                # evacuate on VectorE: ScalarE is saturated by exps, and a
                # stalled evacuation starves the PE long enough to trip the
                # HAM clock gate back to half rate
                nc.vector.tensor_copy(
                    out=vaug_sb[:, st, :, D:2 * D],
                    in_=ps[:].rearrange("p (h d) -> p h d", h=HL),
                )

            # ---- attention, processed in head PAIRS: head 2p sits in array
            # rows 0-63 and head 2p+1 in rows 64-127, so their K=64 QK^T
            # matmuls run CONCURRENTLY in the two row-halves. ----
            def head_views(h0):
                views = []
                for hh in (h0, h0 + 1):
                    jt0, po = hh // 2, (hh % 2) * 64
                    views.append((
                        qT_sb[po:po + 64, jt0, :],
                        kT_sb[po:po + 64, jt0, :],
                        pt_pool.tile([P, ET, S], BF16, tag="pt", name=f"pt_{hh}"),
                    ))
                return views

            def mask_diag(pT, t0, tn):
                # multiply tn diagonal blocks in one strided op: block t sits
                # at free offset t*(S+P) in the flattened [ET, S] tile
                diag = bass.AP(tensor=pT.tensor, offset=pT.offset + t0 * (S + P),
                               ap=[list(pT.ap[0]), [S + P, tn], [1, P]])
                nc.vector.tensor_mul(
                    out=diag, in0=diag,
                    in1=mask_sb[:, None, :].to_broadcast([P, tn, P]),
                )

            def emit_qk_pair(h0):
                views = head_views(h0)
                for t in range(ET):
                    lo = t * P
                    psls = [ps_l.tile([P, 1024], F32, tag="psl", name=f"psl_{t}_{j}") for j in range(2)]
                    for cb in range(2):
                        c0, c1 = cb * 512, (cb + 1) * 512
                        s0 = max(lo, c0)
                        if s0 >= c1:
                            continue
                        # back-to-back row-half matmuls execute concurrently
                        for (qh, kh, _), psl in zip(views, psls):
                            nc.tensor.matmul(
                                psl[:, s0:c1],
                                lhsT=kh[:, lo:lo + P],
                                rhs=qh[:, s0:c1],
                                start=True, stop=True,
                            )
                    for (_, _, pT), psl in zip(views, psls):
                        nc.scalar.activation(
                            out=pT[:, t, lo:S], in_=psl[:, lo:S],
                            func=mybir.ActivationFunctionType.Exp, scale=0.125,
                        )
                for _, _, pT in views:
                    mask_diag(pT, 0, ET)
                return [v[2] for v in views]

            def emit_qk_pair_sqb(views, sqb):
                # last pair, one sq-half at a time: the sq0 exps finish early
                # so the final PV/proj stretch never waits on ScalarE
                c0, c1 = sqb * 512, (sqb + 1) * 512
                for t in range(ET):
                    lo = t * P
                    s0 = max(lo, c0)
                    if s0 >= c1:
                        continue
                    psls = [ps_l.tile([P, 1024], F32, tag="psl",
                                      name=f"psl3_{sqb}_{t}_{j}") for j in range(2)]
                    for (qh, kh, _), psl in zip(views, psls):
                        nc.tensor.matmul(
                            psl[:, s0:c1],
                            lhsT=kh[:, lo:lo + P],
                            rhs=qh[:, s0:c1],
                            start=True, stop=True,
                        )
                    for (_, _, pT), psl in zip(views, psls):
                        nc.scalar.activation(
                            out=pT[:, t, s0:c1], in_=psl[:, s0:c1],
                            func=mybir.ActivationFunctionType.Exp, scale=0.125,
                        )
                for _, _, pT in views:
                    # diag blocks of this half: t in [4*sqb, 4*sqb+4)
                    mask_diag(pT, 4 * sqb, 4)

            def emit_pv(h, pT, sqbs=(0, 1), pso_aps=None):
                jt0, po = h // 2, (h % 2) * 64
                for sqb in sqbs:
                    c0, c1 = sqb * 512, (sqb + 1) * 512
                    if pso_aps is None:
                        pso = ps_o.tile([P, 512], F32, tag="pso")
                    else:
                        pso = pso_aps[sqb]
                    ts = [t for t in range(ET) if t * P < c1]
                    for i, t in enumerate(ts):
                        s0 = max(t * P, c0)
                        off = s0 - c0
                        nc.tensor.matmul(
                            pso[:, off:512],
                            lhsT=vaug_sb[:, t, h, :],
                            rhs=pT[:, t, s0:c1],
                            start=(i == 0), stop=(i == len(ts) - 1),
                            skip_group_check=True,
                        )
                    # normalize: rows 0-63 of pso hold the denominator
                    # (replicated by the ones-half of the stationary operand),
                    # rows 64-127 the unnormalized o2T
                    rec = rec_pool.tile([P, 512], F32)
                    nc.vector.reciprocal_approx_fast(out=rec[:64, :], in_=pso[:64, :])
                    nc.vector.tensor_mul(
                        out=o2T_sb[po:po + 64, jt0, c0:c1],
                        in0=pso[64:128, :], in1=rec[:64, :],
                    )

            def emit_proj(st):
                ob = out_pool.tile([P, 1024], BF16)
                for eb in range(2):
                    psf = ps_mm.tile([P, 512], F32, tag="mm", name=f"mmp_{st}_{eb}")
                    for kt in range(JT):
                        nc.tensor.matmul(
                            psf[:],
                            lhsT=o2T_sb[:, kt, st * P:(st + 1) * P],
                            rhs=wp_sb[:, kt, eb * 512:(eb + 1) * 512],
                            start=(kt == 0), stop=(kt == JT - 1),
                        )
                    # evacuate on ScalarE: it is idle once the last exps are
                    # done, while VectorE still runs the final normalizes
                    nc.scalar.copy(out=ob[:, eb * 512:(eb + 1) * 512], in_=psf[:])
                eng = nc.sync if st % 2 == 0 else nc.gpsimd
                eng.dma_start(out=out[st * P:(st + 1) * P, :], in_=ob[:])

            # ---- master pipeline ----
            emit_phase_a()                     # q/k for jt0+jt1, kt-major
            pair0 = emit_qk_pair(0)            # heads 0,1
            pair1 = emit_qk_pair(2)            # heads 2,3
            emit_qk_chains(2)
            emit_qk_chains(3)
            for st in range(ET):
                emit_v_chain(st)
            emit_pv(0, pair0[0])
            emit_pv(1, pair0[1])
            pair2 = emit_qk_pair(4)            # heads 4,5
            emit_pv(2, pair1[0])
            emit_pv(3, pair1[1])
            views3 = head_views(6)             # heads 6,7
            emit_qk_pair_sqb(views3, 0)
            emit_qk_pair_sqb(views3, 1)
            emit_pv(4, pair2[0])
            emit_pv(5, pair2[1])
            # the last pair's PV accumulators live in the ps_l banks (free
            # after QK3) so they never wait on the ps_o rotation
            l6 = ps_l.tile([P, 1024], F32, tag="psl", name="pv6")
            l7 = ps_l.tile([P, 1024], F32, tag="psl", name="pv7")
            pv6_aps = [l6[:, 0:512], l6[:, 512:1024]]
            pv7_aps = [l7[:, 0:512], l7[:, 512:1024]]
            emit_pv(6, views3[0][2], sqbs=(0,), pso_aps=pv6_aps)
            emit_pv(7, views3[1][2], sqbs=(0,), pso_aps=pv7_aps)
            for st in range(4):
                emit_proj(st)
            emit_pv(6, views3[0][2], sqbs=(1,), pso_aps=pv6_aps)
            emit_pv(7, views3[1][2], sqbs=(1,), pso_aps=pv7_aps)
            for st in range(4, ET):
                emit_proj(st)

    nc.compile()
    return nc


def make_in_maps(x, W_attn, b_attn, W_proj, b_proj):
    bf16 = ml_dtypes.bfloat16
    x = np.asarray(x, dtype=np.float32)
    W_attn = np.asarray(W_attn, dtype=np.float32)
    b_attn = np.asarray(b_attn, dtype=np.float32)
    W_proj = np.asarray(W_proj, dtype=np.float32)
    in_maps = []
    for i in range(NCORES):
        b, g = i // 2, i % 2
        j0 = g * JL
        in_maps.append({
            "xT": np.ascontiguousarray(x[b].T).astype(bf16),
            "wq": W_attn[:, j0:j0 + JL].astype(bf16),
            "wk": W_attn[:, E + j0:E + j0 + JL].astype(bf16),
            "wv": W_attn[:, 2 * E + j0:2 * E + j0 + JL].astype(bf16),
            "wp": W_proj[j0:j0 + JL, :].astype(bf16),
            "bq": np.ascontiguousarray(
                b_attn[j0:j0 + JL].astype(np.float32).reshape(JT, P).T),
            "bk": np.ascontiguousarray(
                b_attn[E + j0:E + j0 + JL].astype(np.float32).reshape(JT, P).T),
        })
    return in_maps


def kernel(x, W_attn, b_attn, W_proj, b_proj):
    global _NC_CACHE
    x = np.asarray(x, dtype=np.float32)
    W_attn = np.asarray(W_attn, dtype=np.float32)
    b_attn = np.asarray(b_attn, dtype=np.float32)
    W_proj = np.asarray(W_proj, dtype=np.float32)
    b_proj = np.asarray(b_proj, dtype=np.float32)

    if _NC_CACHE is None:
        _NC_CACHE = build_nc()
    nc = _NC_CACHE

    in_maps = make_in_maps(x, W_attn, b_attn, W_proj, b_proj)
    res = run_bass_kernel_spmd(nc, in_maps, core_ids=list(range(NCORES)))

    # host unshard: sum the two head-group partials + exact bias corrections
    bias_row = b_proj.copy()
    for g in range(2):
        j0 = g * JL
        bv = b_attn[2 * E + j0:2 * E + j0 + JL].astype(np.float32)
        bias_row += bv @ W_proj[j0:j0 + JL, :].astype(np.float32)

    full = np.empty((B, S, E), np.float32)
    for b in range(B):
        full[b] = (res.results[2 * b]["out"].astype(np.float32)
                   + res.results[2 * b + 1]["out"].astype(np.float32)
                   + bias_row[None, :])
    return full
